# revision 36
# baseline (speedup 1.0000x reference)
"""Angle-feature extraction kernel for 8 TRN2 NeuronCores.

Math (per batch b, atom i):
  a[j,k]   = (d2_ij + d2_ik - d2_jk) / (2 d_ij d_ik)   (0 where den==0)
  fa[i,f]  = sum_jk exp(-100 c_f (a-fp_f)^2) w_ij w_ik / (N*12),  w = cd0 row i

Sharding: 8 cores x 48 (b,i)-rows (cores 0-3: b=0, 4-7: b=1).

Per core, rows are processed in PAIRS (i0,i1): k splits into a
128-partition A chunk per row plus one shared pass stacking both rows'
64-partition B chunks, so every op runs at full partition width. The
integrand is symmetric in j<->k, so the (k in B, j in A) block is skipped
entirely and the mirrored (k in A, j in B) contribution is counted twice
(doubled weights in the final contraction): per-feature width is
192+192+64 = 448 instead of 3*192.

Tiles are [k_partitions, j_free]; -a = (X2*rk - qk).rjb - rk.qjb with
rk/qk per-partition columns and rjb/qjb row-broadcasts fetched from
DMA-replicated tables (no PE involvement). fp/coeff are baked into the
program as immediates (recompiled per parameter set), so the feature
expansion t_f = s_f*(-a) + s_f*fp_f runs at the DVE 2x tensor_scalar
rate. One Square + one Exp (bf16 out) per pair on the scalar engine.
Both weighted contractions are PE matmuls with bf16 operands; partial
sums land in separate PSUM columns (cross-group PSUM accumulation
silently overwrites in this lowering) and a DVE reduce combines them.
Final scale 1/(N*12) rides the output copy.
"""

import sys

sys.path.insert(0, "/opt/trn_rl_repo")

import math
import numpy as np
from contextlib import ExitStack

N = 192
NFA = 5
NCORES = 8
RPC = 48  # (b,i) rows per core
NPAIR = RPC // 2
WF = 2 * N + 64  # 448: per-feature width (pass0 j:192, pass1 j:192, pass2 j:64)
SQRT_HALF = 0.7071067811865476

_BUILT = {}


def _build(sc, bc):
    """sc/bc: per-feature immediates  sc_f = -sqrt(100 c_f), bc_f = sc_f*fp_f."""
    from concourse import bacc, mybir, tile

    F32 = mybir.dt.float32
    BF16 = mybir.dt.bfloat16
    nc = bacc.Bacc(
        "TRN2", target_bir_lowering=False, debug=False, num_devices=NCORES
    )

    d_e = nc.declare_dram_parameter("d", [N, N], F32, isOutput=False)
    di_e = nc.declare_dram_parameter("di", [RPC, N], F32, isOutput=False)
    dsi_e = nc.declare_dram_parameter("dsi", [RPC, N], F32, isOutput=False)
    dit_e = nc.declare_dram_parameter("dit", [N, RPC], F32, isOutput=False)
    dsit_e = nc.declare_dram_parameter("dsit", [N, RPC], F32, isOutput=False)
    cdt_e = nc.declare_dram_parameter("cdt", [N, RPC], F32, isOutput=False)
    out_e = nc.declare_dram_parameter("out", [NFA, RPC], F32, isOutput=True)

    mult = mybir.AluOpType.mult
    subtract = mybir.AluOpType.subtract
    add = mybir.AluOpType.add
    Square = mybir.ActivationFunctionType.Square
    Exp = mybir.ActivationFunctionType.Exp

    with tile.TileContext(nc) as tc, ExitStack() as ctx:
        singles = ctx.enter_context(tc.tile_pool(name="singles", bufs=1))
        work = ctx.enter_context(tc.tile_pool(name="work", bufs=2))
        work4 = ctx.enter_context(tc.tile_pool(name="work4", bufs=4))
        psum2 = ctx.enter_context(tc.tile_pool(name="psum2", bufs=4, space="PSUM"))
        psum3 = ctx.enter_context(tc.tile_pool(name="psum3", bufs=4, space="PSUM"))
        dram = ctx.enter_context(tc.tile_pool(name="dram", bufs=1, space="DRAM"))

        # ---------------- load inputs ----------------
        CHUNKS = [(0, 128), (128, 64)]
        D = {}
        CDT = {}
        for r0, P in CHUNKS:
            t = singles.tile([P, N], F32, tag=f"d_{r0}")
            nc.gpsimd.dma_start(t[:], d_e[r0 : r0 + P, :])
            D[r0] = t
            t = singles.tile([P, RPC], F32, tag=f"cdt_{r0}")
            nc.gpsimd.dma_start(t[:], cdt_e[r0 : r0 + P, :])
            CDT[r0] = t
        dI = singles.tile([RPC, N], F32, tag="dI")
        nc.gpsimd.dma_start(dI[:], di_e[:])
        dsI = singles.tile([RPC, N], F32, tag="dsI")
        nc.gpsimd.dma_start(dsI[:], dsi_e[:])
        dIT, dsIT = {}, {}
        for r0, P in CHUNKS:
            t = singles.tile([P, RPC], F32, tag=f"dit_{r0}")
            nc.gpsimd.dma_start(t[:], dit_e[r0 : r0 + P, :])
            dIT[r0] = t
            t = singles.tile([P, RPC], F32, tag=f"dsit_{r0}")
            nc.gpsimd.dma_start(t[:], dsit_e[r0 : r0 + P, :])
            dsIT[r0] = t

        # ---------------- derived matrices ----------------
        X2 = {}
        for r0, P in CHUNKS:
            t = singles.tile([P, N], F32, tag=f"x2_{r0}")
            nc.vector.tensor_mul(t[:], D[r0][:], D[r0][:])
            X2[r0] = t
        RI = singles.tile([RPC, N], F32, tag="RI")
        nc.vector.reciprocal(RI[:], dsI[:])
        nc.vector.tensor_scalar_mul(RI[:], RI[:], SQRT_HALF)
        QI = singles.tile([RPC, N], F32, tag="QI")
        nc.vector.tensor_mul(QI[:], dI[:], dI[:])
        nc.vector.tensor_mul(QI[:], QI[:], RI[:])

        # rk/qk per-partition column tables
        RIT = {}
        QIT = {}
        for r0, P in CHUNKS:
            rt = singles.tile([P, RPC], F32, tag=f"rit_{r0}")
            nc.vector.reciprocal(rt[:], dsIT[r0][:])
            nc.vector.tensor_scalar_mul(rt[:], rt[:], SQRT_HALF)
            RIT[r0] = rt
            qt = singles.tile([P, RPC], F32, tag=f"qit_{r0}")
            nc.vector.tensor_mul(qt[:], dIT[r0][:], dIT[r0][:])
            nc.vector.tensor_mul(qt[:], qt[:], rt[:])
            QIT[r0] = qt

        # rjb/qjb row-broadcast tables, DMA-replicated via a DRAM bounce.
        # RJB[p, i*192+j] = RI[i, j] for every partition p; the pair tables
        # RJB3 stack even rows on partitions 0-63 and odd rows on 64-127,
        # restricted to j in [128,192).
        HR = RPC * N // 2
        RJB = [
            singles.tile([128, HR], F32, tag="RJBa", name="RJBa"),
            singles.tile([128, HR], F32, tag="RJBb", name="RJBb"),
        ]
        QJB = [
            singles.tile([128, HR], F32, tag="QJBa", name="QJBa"),
            singles.tile([128, HR], F32, tag="QJBb", name="QJBb"),
        ]
        RJB3 = singles.tile([128, NPAIR * 64], F32, tag="RJB3")
        QJB3 = singles.tile([128, NPAIR * 64], F32, tag="QJB3")
        for src, dst, dst3, nm in ((RI, RJB, RJB3, "ri"), (QI, QJB, QJB3, "qi")):
            scr = dram.tile([1, RPC * N], F32, tag=f"scr_{nm}")
            nc.gpsimd.dma_start(
                scr[:].rearrange("a (r c) -> (a r) c", r=RPC, c=N), src[:]
            )
            for h in range(2):
                nc.gpsimd.dma_start(
                    dst[h][:], scr[:, h * HR : (h + 1) * HR].broadcast_to([128, HR])
                )
            v = scr[:].rearrange("a (q two c) -> a q two c", q=NPAIR, two=2, c=N)
            nc.gpsimd.dma_start(
                dst3[0:64, :],
                v[:, :, 0:1, 128:N].broadcast_to([64, NPAIR, 1, 64]),
            )
            nc.gpsimd.dma_start(
                dst3[64:128, :],
                v[:, :, 1:2, 128:N].broadcast_to([64, NPAIR, 1, 64]),
            )

        # bf16 weight columns for the PE contractions (+ doubled B weights
        # standing in for the skipped mirror block)
        CDTb = {}
        for r0, P in CHUNKS:
            t = singles.tile([P, RPC], BF16, tag=f"cdtb_{r0}")
            nc.vector.tensor_copy(t[:], CDT[r0][:])
            CDTb[r0] = t
        CDTb2x = singles.tile([64, RPC], BF16, tag="cdtb2x")
        nc.vector.tensor_scalar_mul(CDTb2x[:], CDT[128][:], 2.0)

        # stacked-pair tables for the shared B pass
        X2B2 = singles.tile([128, N], F32, tag="x2b2")
        nc.gpsimd.dma_start(X2B2[0:64, :], X2[128][:])
        nc.gpsimd.dma_start(X2B2[64:128, :], X2[128][:])
        RIT3 = singles.tile([128, NPAIR], F32, tag="rit3")
        QIT3 = singles.tile([128, NPAIR], F32, tag="qit3")
        CDT3b = singles.tile([128, NPAIR], BF16, tag="cdt3b")
        for dst, srct in ((RIT3, RIT[128]), (QIT3, QIT[128]), (CDT3b, CDTb[128])):
            s3 = srct[:].rearrange("p (i two) -> p i two", two=2)
            nc.gpsimd.dma_start(dst[0:64, :], s3[:, :, 0:1])
            nc.gpsimd.dma_start(dst[64:128, :], s3[:, :, 1:2])

        FA = singles.tile([NFA, RPC], F32, tag="FA")

        # ---------------- main loop: 12 super-iterations of 2 row pairs ----
        # Software-pipelined: contractions for super-iter s are emitted after
        # the a-build/T5 of super-iter s+1, keeping ACT fed.
        PW = NFA * WF  # 2240: one pair's feature block
        pending = None  # (E, pairs) awaiting contraction

        def contract(E, pairs):
            for u, p in enumerate(pairs):
                i0, i1 = 2 * p, 2 * p + 1
                eo = u * PW
                vvs = {}
                for i in (i0, i1):
                    vvs[i] = psum3.tile(
                        [128, 3 * NFA], F32, tag="vps", name=f"vv_{i}"
                    )
                for f in range(NFA):
                    for t, i in ((0, i0), (1, i1)):
                        wk = CDTb[0][:, i : i + 1]
                        c = eo + f * WF + t * N
                        nc.tensor.matmul(
                            vvs[i][0:128, f : f + 1], E[:, c : c + 128], wk
                        )
                        nc.tensor.matmul(
                            vvs[i][0:64, NFA + f : NFA + f + 1],
                            E[:, c + 128 : c + N],
                            wk,
                        )
                    c = eo + f * WF + 2 * N
                    for h, i in enumerate((i0, i1)):
                        nc.tensor.matmul(
                            vvs[i][0:64, 2 * NFA + f : 2 * NFA + f + 1],
                            E[64 * h : 64 * (h + 1), c : c + 64],
                            CDT3b[64 * h : 64 * (h + 1), p : p + 1],
                        )
                for i in (i0, i1):
                    V5 = work4.tile(
                        [128, 3 * NFA], BF16, tag="V5", name=f"V5_{i}"
                    )
                    nc.vector.tensor_copy(V5[:], vvs[i][:])
                    fa3 = psum2.tile([NFA, 3], F32, tag="fa3", name=f"fa3_{i}")
                    nc.tensor.matmul(
                        fa3[:, 0:1], V5[0:128, 0:NFA], CDTb[0][:, i : i + 1]
                    )
                    nc.tensor.matmul(
                        fa3[:, 1:2], V5[0:64, NFA : 2 * NFA], CDTb2x[:, i : i + 1]
                    )
                    nc.tensor.matmul(
                        fa3[:, 2:3],
                        V5[0:64, 2 * NFA : 3 * NFA],
                        CDTb[128][:, i : i + 1],
                    )
                    nc.vector.tensor_reduce(
                        FA[:, i : i + 1],
                        fa3[:],
                        mybir.AxisListType.X,
                        mybir.AluOpType.add,
                    )

        for spi in range(NPAIR // 2):
            pairs = (2 * spi, 2 * spi + 1)

            T5 = work.tile([128, 2 * PW], BF16, tag="T5")
            for u, p in enumerate(pairs):
                i0, i1 = 2 * p, 2 * p + 1
                V3 = work4.tile([128, WF], F32, tag="V3", name=f"V3_{p}")
                Am3 = work4.tile([128, WF], F32, tag="Am3", name=f"Am3_{p}")

                for t in range(3):
                    if t < 2:
                        i = (i0, i1)[t]
                        h, base = i // (RPC // 2), (i % (RPC // 2)) * N
                        X2ap = X2[0][:]
                        rk = RIT[0][:, i : i + 1]
                        qk = QIT[0][:, i : i + 1]
                        rjb = RJB[h][:, base : base + N]
                        qjb = QJB[h][:, base : base + N]
                        off, w = t * N, N
                    else:
                        X2ap = X2B2[:, 128:N]
                        rk = RIT3[:, p : p + 1]
                        qk = QIT3[:, p : p + 1]
                        rjb = RJB3[:, p * 64 : (p + 1) * 64]
                        qjb = QJB3[:, p * 64 : (p + 1) * 64]
                        off, w = 2 * N, 64
                    Vs = V3[:, off : off + w]
                    As = Am3[:, off : off + w]
                    nc.gpsimd.tensor_scalar(Vs, X2ap, rk, qk, mult, subtract)
                    nc.vector.tensor_mul(As, Vs, rjb)
                    # As = (qjb*rk) - V*rjb = +a
                    nc.vector.scalar_tensor_tensor(As, qjb, rk, As, mult, subtract)

                for f in range(NFA):
                    nc.vector.tensor_scalar(
                        T5[:, u * PW + f * WF : u * PW + (f + 1) * WF],
                        Am3[:],
                        sc[f],
                        bc[f],
                        mult,
                        add,
                    )

            nc.vector.tensor_mul(T5[:], T5[:], T5[:])  # bf16 2x square
            E = work.tile([128, 2 * PW], BF16, tag="E")
            nc.scalar.activation(E[:], T5[:], Exp, scale=-1.0)

            if pending is not None:
                contract(*pending)
            pending = (E, pairs)

        contract(*pending)

        outs = singles.tile([NFA, RPC], F32, tag="outs")
        nc.scalar.mul(outs[:], FA[:], 1.0 / (N * 12))
        nc.gpsimd.dma_start(out_e[:], outs[:])

    nc.finalize()
    return nc


def _get_nc(fp5, c5):
    key = (tuple(np.asarray(fp5).ravel().tolist()), tuple(np.asarray(c5).ravel().tolist()))
    if key not in _BUILT:
        # Am3 holds +a, so t_f = sqrt(100 c_f) * a - sqrt(100 c_f) * fp_f
        sc = [math.sqrt(100.0 * float(c)) for c in np.asarray(c5).ravel()]
        bc = [-s * float(f) for s, f in zip(sc, np.asarray(fp5).ravel())]
        _BUILT[key] = _build(sc, bc)
    return _BUILT[key]


def kernel(d, cd, fp, coeff):
    from concourse.bass_utils import run_bass_kernel_spmd

    d = np.asarray(d, dtype=np.float32)
    cd = np.asarray(cd, dtype=np.float32)
    cd0 = np.where(cd == 1.0, 0.0, cd).astype(np.float32)
    fp5 = np.asarray(fp, dtype=np.float32).reshape(NFA)
    c5 = np.asarray(coeff, dtype=np.float32).reshape(NFA)
    eye = np.eye(N, dtype=np.float32)

    in_maps = []
    for c in range(NCORES):
        b, i0 = c // 4, RPC * (c % 4)
        ds = d[b] + eye
        in_maps.append(
            {
                "d": np.ascontiguousarray(d[b]),
                "di": np.ascontiguousarray(d[b][i0 : i0 + RPC, :]),
                "dsi": np.ascontiguousarray(ds[i0 : i0 + RPC, :]),
                "dit": np.ascontiguousarray(d[b][:, i0 : i0 + RPC]),
                "dsit": np.ascontiguousarray(ds[:, i0 : i0 + RPC]),
                "cdt": np.ascontiguousarray(cd0[b].T[:, i0 : i0 + RPC]),
            }
        )

    global _last_in_maps, _last_res
    _last_in_maps = in_maps
    nc = _get_nc(fp5, c5)
    res = run_bass_kernel_spmd(nc, in_maps, core_ids=list(range(NCORES)))
    _last_res = res

    fa = np.zeros((2, N, NFA), np.float32)
    for c in range(NCORES):
        b, i0 = c // 4, RPC * (c % 4)
        fa[b, i0 : i0 + RPC, :] = res.results[c]["out"].T
    return fa


# revision 38
# speedup vs baseline: 1.5011x; 1.5011x over previous
"""Angle-feature extraction kernel for 8 TRN2 NeuronCores.

Math (per batch b, atom i):
  a[j,k]   = (d2_ij + d2_ik - d2_jk) / (2 d_ij d_ik)   (0 where den==0)
  fa[i,f]  = sum_jk exp(-100 c_f (a-fp_f)^2) w_ij w_ik / (N*12),  w = cd0 row i

Sharding: 8 cores x 48 (b,i)-rows (cores 0-3: b=0, 4-7: b=1).

Per core, rows are processed in PAIRS (i0,i1): k splits into a
128-partition A chunk per row plus one shared pass stacking both rows'
64-partition B chunks, so every op runs at full partition width. The
integrand is symmetric in j<->k, so the (k in B, j in A) block is skipped
entirely and the mirrored (k in A, j in B) contribution is counted twice
(doubled weights in the final contraction): per-feature width is
192+192+64 = 448 instead of 3*192.

Tiles are [k_partitions, j_free]; -a = (X2*rk - qk).rjb - rk.qjb with
rk/qk per-partition columns and rjb/qjb row-broadcasts fetched from
DMA-replicated tables (no PE involvement). fp/coeff are baked into the
program as immediates (recompiled per parameter set), so the feature
expansion t_f = s_f*(-a) + s_f*fp_f runs at the DVE 2x tensor_scalar
rate. One Square + one Exp (bf16 out) per pair on the scalar engine.
Both weighted contractions are PE matmuls with bf16 operands; partial
sums land in separate PSUM columns (cross-group PSUM accumulation
silently overwrites in this lowering) and a DVE reduce combines them.
Final scale 1/(N*12) rides the output copy.
"""

import sys

sys.path.insert(0, "/opt/trn_rl_repo")

import math
import numpy as np
from contextlib import ExitStack

N = 192
NFA = 5
NCORES = 8
RPC = 48  # (b,i) rows per core
NPAIR = RPC // 2
WF = 2 * N + 64  # 448: per-feature width (pass0 j:192, pass1 j:192, pass2 j:64)
SQRT_HALF = 0.7071067811865476

_BUILT = {}


def _build(sc, bc):
    """sc/bc: per-feature immediates  sc_f = -sqrt(100 c_f), bc_f = sc_f*fp_f."""
    from concourse import bacc, mybir, tile

    F32 = mybir.dt.float32
    BF16 = mybir.dt.bfloat16
    nc = bacc.Bacc(
        "TRN2", target_bir_lowering=False, debug=False, num_devices=NCORES
    )

    d_e = nc.declare_dram_parameter("d", [N, N], F32, isOutput=False)
    di_e = nc.declare_dram_parameter("di", [RPC, N], F32, isOutput=False)
    dsi_e = nc.declare_dram_parameter("dsi", [RPC, N], F32, isOutput=False)
    dit_e = nc.declare_dram_parameter("dit", [N, RPC], F32, isOutput=False)
    dsit_e = nc.declare_dram_parameter("dsit", [N, RPC], F32, isOutput=False)
    cdt_e = nc.declare_dram_parameter("cdt", [N, RPC], F32, isOutput=False)
    out_e = nc.declare_dram_parameter("out", [NFA, RPC], F32, isOutput=True)

    mult = mybir.AluOpType.mult
    subtract = mybir.AluOpType.subtract
    add = mybir.AluOpType.add
    Square = mybir.ActivationFunctionType.Square
    Exp = mybir.ActivationFunctionType.Exp
    Identity = mybir.ActivationFunctionType.Identity

    with tile.TileContext(nc) as tc, ExitStack() as ctx:
        singles = ctx.enter_context(tc.tile_pool(name="singles", bufs=1))
        work = ctx.enter_context(tc.tile_pool(name="work", bufs=2))
        work4 = ctx.enter_context(tc.tile_pool(name="work4", bufs=4))
        psum2 = ctx.enter_context(tc.tile_pool(name="psum2", bufs=4, space="PSUM"))
        psum3 = ctx.enter_context(tc.tile_pool(name="psum3", bufs=4, space="PSUM"))
        dram = ctx.enter_context(tc.tile_pool(name="dram", bufs=1, space="DRAM"))

        # ---------------- load inputs ----------------
        CHUNKS = [(0, 128), (128, 64)]
        D = {}
        CDT = {}
        for r0, P in CHUNKS:
            t = singles.tile([P, N], F32, tag=f"d_{r0}")
            nc.gpsimd.dma_start(t[:], d_e[r0 : r0 + P, :])
            D[r0] = t
            t = singles.tile([P, RPC], F32, tag=f"cdt_{r0}")
            nc.gpsimd.dma_start(t[:], cdt_e[r0 : r0 + P, :])
            CDT[r0] = t
        dI = singles.tile([RPC, N], F32, tag="dI")
        nc.gpsimd.dma_start(dI[:], di_e[:])
        dsI = singles.tile([RPC, N], F32, tag="dsI")
        nc.gpsimd.dma_start(dsI[:], dsi_e[:])
        dIT, dsIT = {}, {}
        for r0, P in CHUNKS:
            t = singles.tile([P, RPC], F32, tag=f"dit_{r0}")
            nc.gpsimd.dma_start(t[:], dit_e[r0 : r0 + P, :])
            dIT[r0] = t
            t = singles.tile([P, RPC], F32, tag=f"dsit_{r0}")
            nc.gpsimd.dma_start(t[:], dsit_e[r0 : r0 + P, :])
            dsIT[r0] = t

        # ---------------- derived matrices ----------------
        X2 = {}
        for r0, P in CHUNKS:
            t = singles.tile([P, N], F32, tag=f"x2_{r0}")
            nc.vector.tensor_mul(t[:], D[r0][:], D[r0][:])
            X2[r0] = t
        RI = singles.tile([RPC, N], F32, tag="RI")
        nc.vector.reciprocal(RI[:], dsI[:])
        nc.vector.tensor_scalar_mul(RI[:], RI[:], SQRT_HALF)
        QI = singles.tile([RPC, N], F32, tag="QI")
        nc.vector.tensor_mul(QI[:], dI[:], dI[:])
        nc.vector.tensor_mul(QI[:], QI[:], RI[:])

        # rk/qk per-partition column tables
        RIT = {}
        QIT = {}
        for r0, P in CHUNKS:
            rt = singles.tile([P, RPC], F32, tag=f"rit_{r0}")
            nc.vector.reciprocal(rt[:], dsIT[r0][:])
            nc.vector.tensor_scalar_mul(rt[:], rt[:], SQRT_HALF)
            RIT[r0] = rt
            qt = singles.tile([P, RPC], F32, tag=f"qit_{r0}")
            nc.vector.tensor_mul(qt[:], dIT[r0][:], dIT[r0][:])
            nc.vector.tensor_mul(qt[:], qt[:], rt[:])
            QIT[r0] = qt

        # rjb/qjb row-broadcast tables, DMA-replicated via a DRAM bounce.
        # RJB[p, i*192+j] = RI[i, j] for every partition p; the pair tables
        # RJB3 stack even rows on partitions 0-63 and odd rows on 64-127,
        # restricted to j in [128,192).
        HR = RPC * N // 2
        RJB = [
            singles.tile([128, HR], F32, tag="RJBa", name="RJBa"),
            singles.tile([128, HR], F32, tag="RJBb", name="RJBb"),
        ]
        QJB = [
            singles.tile([128, HR], F32, tag="QJBa", name="QJBa"),
            singles.tile([128, HR], F32, tag="QJBb", name="QJBb"),
        ]
        RJB3 = singles.tile([128, NPAIR * 64], F32, tag="RJB3")
        QJB3 = singles.tile([128, NPAIR * 64], F32, tag="QJB3")
        for src, dst, dst3, nm in ((RI, RJB, RJB3, "ri"), (QI, QJB, QJB3, "qi")):
            scr = dram.tile([1, RPC * N], F32, tag=f"scr_{nm}")
            nc.gpsimd.dma_start(
                scr[:].rearrange("a (r c) -> (a r) c", r=RPC, c=N), src[:]
            )
            for h in range(2):
                nc.gpsimd.dma_start(
                    dst[h][:], scr[:, h * HR : (h + 1) * HR].broadcast_to([128, HR])
                )
            v = scr[:].rearrange("a (q two c) -> a q two c", q=NPAIR, two=2, c=N)
            nc.gpsimd.dma_start(
                dst3[0:64, :],
                v[:, :, 0:1, 128:N].broadcast_to([64, NPAIR, 1, 64]),
            )
            nc.gpsimd.dma_start(
                dst3[64:128, :],
                v[:, :, 1:2, 128:N].broadcast_to([64, NPAIR, 1, 64]),
            )

        # bf16 weight columns for the PE contractions (+ doubled B weights
        # standing in for the skipped mirror block)
        CDTb = {}
        for r0, P in CHUNKS:
            t = singles.tile([P, RPC], BF16, tag=f"cdtb_{r0}")
            nc.vector.tensor_copy(t[:], CDT[r0][:])
            CDTb[r0] = t
        CDTb2x = singles.tile([64, RPC], BF16, tag="cdtb2x")
        nc.vector.tensor_scalar_mul(CDTb2x[:], CDT[128][:], 2.0)

        # stacked-pair tables for the shared B pass
        X2B2 = singles.tile([128, N], F32, tag="x2b2")
        nc.gpsimd.dma_start(X2B2[0:64, :], X2[128][:])
        nc.gpsimd.dma_start(X2B2[64:128, :], X2[128][:])
        RIT3 = singles.tile([128, NPAIR], F32, tag="rit3")
        QIT3 = singles.tile([128, NPAIR], F32, tag="qit3")
        CDT3b = singles.tile([128, NPAIR], BF16, tag="cdt3b")
        for dst, srct in ((RIT3, RIT[128]), (QIT3, QIT[128]), (CDT3b, CDTb[128])):
            s3 = srct[:].rearrange("p (i two) -> p i two", two=2)
            nc.gpsimd.dma_start(dst[0:64, :], s3[:, :, 0:1])
            nc.gpsimd.dma_start(dst[64:128, :], s3[:, :, 1:2])
        QITn0 = singles.tile([128, RPC], F32, tag="qitn0")
        nc.vector.tensor_scalar_mul(QITn0[:], QIT[0][:], -1.0)
        QIT3n = singles.tile([128, NPAIR], F32, tag="qit3n")
        nc.vector.tensor_scalar_mul(QIT3n[:], QIT3[:], -1.0)

        FA = singles.tile([NFA, RPC], F32, tag="FA")

        # ---------------- main loop: 12 super-iterations of 2 row pairs ----
        # Software-pipelined: contractions for super-iter s are emitted after
        # the a-build/T5 of super-iter s+1, keeping ACT fed.
        PW = NFA * WF  # 2240: one pair's feature block
        pending = None  # (E, pairs) awaiting contraction

        def contract(E, pairs):
            for u, p in enumerate(pairs):
                i0, i1 = 2 * p, 2 * p + 1
                eo = u * PW
                vvs = {}
                for i in (i0, i1):
                    vvs[i] = psum3.tile(
                        [128, 3 * NFA], F32, tag="vps", name=f"vv_{i}"
                    )
                for f in range(NFA):
                    for t, i in ((0, i0), (1, i1)):
                        wk = CDTb[0][:, i : i + 1]
                        c = eo + f * WF + t * N
                        nc.tensor.matmul(
                            vvs[i][0:128, f : f + 1], E[:, c : c + 128], wk
                        )
                        nc.tensor.matmul(
                            vvs[i][0:64, NFA + f : NFA + f + 1],
                            E[:, c + 128 : c + N],
                            wk,
                        )
                    c = eo + f * WF + 2 * N
                    for h, i in enumerate((i0, i1)):
                        nc.tensor.matmul(
                            vvs[i][0:64, 2 * NFA + f : 2 * NFA + f + 1],
                            E[64 * h : 64 * (h + 1), c : c + 64],
                            CDT3b[64 * h : 64 * (h + 1), p : p + 1],
                        )
                for i in (i0, i1):
                    V5 = work4.tile(
                        [128, 3 * NFA], BF16, tag="V5", name=f"V5_{i}"
                    )
                    nc.scalar.copy(V5[:], vvs[i][:])
                    fa3 = psum2.tile([NFA, 3], F32, tag="fa3", name=f"fa3_{i}")
                    nc.tensor.matmul(
                        fa3[:, 0:1], V5[0:128, 0:NFA], CDTb[0][:, i : i + 1]
                    )
                    nc.tensor.matmul(
                        fa3[:, 1:2], V5[0:64, NFA : 2 * NFA], CDTb2x[:, i : i + 1]
                    )
                    nc.tensor.matmul(
                        fa3[:, 2:3],
                        V5[0:64, 2 * NFA : 3 * NFA],
                        CDTb[128][:, i : i + 1],
                    )
                    nc.vector.tensor_reduce(
                        FA[:, i : i + 1],
                        fa3[:],
                        mybir.AxisListType.X,
                        mybir.AluOpType.add,
                    )

        for spi in range(NPAIR // 2):
            pairs = (2 * spi, 2 * spi + 1)

            T5 = work.tile([128, 2 * PW], BF16, tag="T5")
            for u, p in enumerate(pairs):
                i0, i1 = 2 * p, 2 * p + 1
                V3 = work4.tile([128, WF], F32, tag="V3", name=f"V3_{p}")
                Am3 = work4.tile([128, WF], F32, tag="Am3", name=f"Am3_{p}")

                for t in range(3):
                    if t < 2:
                        i = (i0, i1)[t]
                        h, base = i // (RPC // 2), (i % (RPC // 2)) * N
                        X2ap = X2[0][:]
                        rk = RIT[0][:, i : i + 1]
                        qkn = QITn0[:, i : i + 1]
                        rjb = RJB[h][:, base : base + N]
                        qjb = QJB[h][:, base : base + N]
                        off, w = t * N, N
                    else:
                        X2ap = X2B2[:, 128:N]
                        rk = RIT3[:, p : p + 1]
                        qkn = QIT3n[:, p : p + 1]
                        rjb = RJB3[:, p * 64 : (p + 1) * 64]
                        qjb = QJB3[:, p * 64 : (p + 1) * 64]
                        off, w = 2 * N, 64
                    Vs = V3[:, off : off + w]
                    As = Am3[:, off : off + w]
                    # V = rk*X2 - qk on ACT (Identity with per-partition
                    # scale/bias); qkn = -qk column
                    nc.scalar.activation(Vs, X2ap, Identity, bias=qkn, scale=rk)
                    nc.vector.tensor_mul(As, Vs, rjb)
                    # As = (qjb*rk) - V*rjb = +a
                    nc.vector.scalar_tensor_tensor(As, qjb, rk, As, mult, subtract)

                for f in range(NFA):
                    nc.vector.tensor_scalar(
                        T5[:, u * PW + f * WF : u * PW + (f + 1) * WF],
                        Am3[:],
                        sc[f],
                        bc[f],
                        mult,
                        add,
                    )

            nc.scalar.activation(T5[:], T5[:], Square)  # bf16 in/out on ACT
            E = work.tile([128, 2 * PW], BF16, tag="E")
            nc.scalar.activation(E[:], T5[:], Exp, scale=-1.0)

            if pending is not None:
                contract(*pending)
            pending = (E, pairs)

        contract(*pending)

        outs = singles.tile([NFA, RPC], F32, tag="outs")
        nc.scalar.mul(outs[:], FA[:], 1.0 / (N * 12))
        nc.gpsimd.dma_start(out_e[:], outs[:])

    nc.finalize()
    return nc


def _get_nc(fp5, c5):
    key = (tuple(np.asarray(fp5).ravel().tolist()), tuple(np.asarray(c5).ravel().tolist()))
    if key not in _BUILT:
        # Am3 holds +a, so t_f = sqrt(100 c_f) * a - sqrt(100 c_f) * fp_f
        sc = [math.sqrt(100.0 * float(c)) for c in np.asarray(c5).ravel()]
        bc = [-s * float(f) for s, f in zip(sc, np.asarray(fp5).ravel())]
        _BUILT[key] = _build(sc, bc)
    return _BUILT[key]


def kernel(d, cd, fp, coeff):
    from concourse.bass_utils import run_bass_kernel_spmd

    d = np.asarray(d, dtype=np.float32)
    cd = np.asarray(cd, dtype=np.float32)
    cd0 = np.where(cd == 1.0, 0.0, cd).astype(np.float32)
    fp5 = np.asarray(fp, dtype=np.float32).reshape(NFA)
    c5 = np.asarray(coeff, dtype=np.float32).reshape(NFA)
    eye = np.eye(N, dtype=np.float32)

    in_maps = []
    for c in range(NCORES):
        b, i0 = c // 4, RPC * (c % 4)
        ds = d[b] + eye
        in_maps.append(
            {
                "d": np.ascontiguousarray(d[b]),
                "di": np.ascontiguousarray(d[b][i0 : i0 + RPC, :]),
                "dsi": np.ascontiguousarray(ds[i0 : i0 + RPC, :]),
                "dit": np.ascontiguousarray(d[b][:, i0 : i0 + RPC]),
                "dsit": np.ascontiguousarray(ds[:, i0 : i0 + RPC]),
                "cdt": np.ascontiguousarray(cd0[b].T[:, i0 : i0 + RPC]),
            }
        )

    global _last_in_maps, _last_res
    _last_in_maps = in_maps
    nc = _get_nc(fp5, c5)
    res = run_bass_kernel_spmd(nc, in_maps, core_ids=list(range(NCORES)))
    _last_res = res

    fa = np.zeros((2, N, NFA), np.float32)
    for c in range(NCORES):
        b, i0 = c // 4, RPC * (c % 4)
        fa[b, i0 : i0 + RPC, :] = res.results[c]["out"].T
    return fa


# revision 40
# speedup vs baseline: 1.6206x; 1.0796x over previous
"""Angle-feature extraction kernel for 8 TRN2 NeuronCores.

Math (per batch b, atom i):
  a[j,k]   = (d2_ij + d2_ik - d2_jk) / (2 d_ij d_ik)   (0 where den==0)
  fa[i,f]  = sum_jk exp(-100 c_f (a-fp_f)^2) w_ij w_ik / (N*12),  w = cd0 row i

Sharding: 8 cores x 48 (b,i)-rows (cores 0-3: b=0, 4-7: b=1).

Per core, rows are processed in PAIRS (i0,i1): k splits into a
128-partition A chunk per row plus one shared pass stacking both rows'
64-partition B chunks, so every op runs at full partition width. The
integrand is symmetric in j<->k, so the (k in B, j in A) block is skipped
entirely and the mirrored (k in A, j in B) contribution is counted twice
(doubled weights in the final contraction): per-feature width is
192+192+64 = 448 instead of 3*192.

Tiles are [k_partitions, j_free]; -a = (X2*rk - qk).rjb - rk.qjb with
rk/qk per-partition columns and rjb/qjb row-broadcasts fetched from
DMA-replicated tables (no PE involvement). fp/coeff are baked into the
program as immediates (recompiled per parameter set), so the feature
expansion t_f = s_f*(-a) + s_f*fp_f runs at the DVE 2x tensor_scalar
rate. One Square + one Exp (bf16 out) per pair on the scalar engine.
Both weighted contractions are PE matmuls with bf16 operands; partial
sums land in separate PSUM columns (cross-group PSUM accumulation
silently overwrites in this lowering) and a DVE reduce combines them.
Final scale 1/(N*12) rides the output copy.
"""

import sys

sys.path.insert(0, "/opt/trn_rl_repo")

import math
import numpy as np
from contextlib import ExitStack

N = 192
NFA = 5
NCORES = 8
RPC = 48  # (b,i) rows per core
NPAIR = RPC // 2
WF = 2 * N + 64  # 448: per-feature width (pass0 j:192, pass1 j:192, pass2 j:64)
SQRT_HALF = 0.7071067811865476

_BUILT = {}


def _build(sc, bc):
    """sc/bc: per-feature immediates  sc_f = -sqrt(100 c_f), bc_f = sc_f*fp_f."""
    from concourse import bacc, mybir, tile

    F32 = mybir.dt.float32
    BF16 = mybir.dt.bfloat16
    nc = bacc.Bacc(
        "TRN2", target_bir_lowering=False, debug=False, num_devices=NCORES
    )

    d_e = nc.declare_dram_parameter("d", [N, N], F32, isOutput=False)
    di_e = nc.declare_dram_parameter("di", [RPC, N], F32, isOutput=False)
    dsi_e = nc.declare_dram_parameter("dsi", [RPC, N], F32, isOutput=False)
    dit_e = nc.declare_dram_parameter("dit", [N, RPC], F32, isOutput=False)
    dsit_e = nc.declare_dram_parameter("dsit", [N, RPC], F32, isOutput=False)
    cdt_e = nc.declare_dram_parameter("cdt", [N, RPC], F32, isOutput=False)
    out_e = nc.declare_dram_parameter("out", [NFA, RPC], F32, isOutput=True)

    mult = mybir.AluOpType.mult
    subtract = mybir.AluOpType.subtract
    add = mybir.AluOpType.add
    Square = mybir.ActivationFunctionType.Square
    Exp = mybir.ActivationFunctionType.Exp
    Identity = mybir.ActivationFunctionType.Identity

    with tile.TileContext(nc) as tc, ExitStack() as ctx:
        singles = ctx.enter_context(tc.tile_pool(name="singles", bufs=1))
        work = ctx.enter_context(tc.tile_pool(name="work", bufs=3))
        work4 = ctx.enter_context(tc.tile_pool(name="work4", bufs=4))
        psum2 = ctx.enter_context(tc.tile_pool(name="psum2", bufs=4, space="PSUM"))
        psum3 = ctx.enter_context(tc.tile_pool(name="psum3", bufs=4, space="PSUM"))
        dram = ctx.enter_context(tc.tile_pool(name="dram", bufs=1, space="DRAM"))

        # ---------------- load inputs ----------------
        CHUNKS = [(0, 128), (128, 64)]
        D = {}
        CDT = {}
        for r0, P in CHUNKS:
            t = singles.tile([P, N], F32, tag=f"d_{r0}")
            nc.gpsimd.dma_start(t[:], d_e[r0 : r0 + P, :])
            D[r0] = t
            t = singles.tile([P, RPC], F32, tag=f"cdt_{r0}")
            nc.gpsimd.dma_start(t[:], cdt_e[r0 : r0 + P, :])
            CDT[r0] = t
        dI = singles.tile([RPC, N], F32, tag="dI")
        nc.gpsimd.dma_start(dI[:], di_e[:])
        dsI = singles.tile([RPC, N], F32, tag="dsI")
        nc.gpsimd.dma_start(dsI[:], dsi_e[:])
        dIT, dsIT = {}, {}
        for r0, P in CHUNKS:
            t = singles.tile([P, RPC], F32, tag=f"dit_{r0}")
            nc.gpsimd.dma_start(t[:], dit_e[r0 : r0 + P, :])
            dIT[r0] = t
            t = singles.tile([P, RPC], F32, tag=f"dsit_{r0}")
            nc.gpsimd.dma_start(t[:], dsit_e[r0 : r0 + P, :])
            dsIT[r0] = t

        # ---------------- derived matrices ----------------
        X2 = {}
        for r0, P in CHUNKS:
            t = singles.tile([P, N], F32, tag=f"x2_{r0}")
            nc.vector.tensor_mul(t[:], D[r0][:], D[r0][:])
            X2[r0] = t
        RI = singles.tile([RPC, N], F32, tag="RI")
        nc.vector.reciprocal(RI[:], dsI[:])
        nc.vector.tensor_scalar_mul(RI[:], RI[:], SQRT_HALF)
        QI = singles.tile([RPC, N], F32, tag="QI")
        nc.vector.tensor_mul(QI[:], dI[:], dI[:])
        nc.vector.tensor_mul(QI[:], QI[:], RI[:])

        # rk/qk per-partition column tables
        RIT = {}
        QIT = {}
        for r0, P in CHUNKS:
            rt = singles.tile([P, RPC], F32, tag=f"rit_{r0}")
            nc.vector.reciprocal(rt[:], dsIT[r0][:])
            nc.vector.tensor_scalar_mul(rt[:], rt[:], SQRT_HALF)
            RIT[r0] = rt
            qt = singles.tile([P, RPC], F32, tag=f"qit_{r0}")
            nc.vector.tensor_mul(qt[:], dIT[r0][:], dIT[r0][:])
            nc.vector.tensor_mul(qt[:], qt[:], rt[:])
            QIT[r0] = qt

        # rjb/qjb row-broadcast tables, DMA-replicated via a DRAM bounce.
        # RJB[p, i*192+j] = RI[i, j] for every partition p; the pair tables
        # RJB3 stack even rows on partitions 0-63 and odd rows on 64-127,
        # restricted to j in [128,192).
        HR = RPC * N // 2
        RJB = [
            singles.tile([128, HR], F32, tag="RJBa", name="RJBa"),
            singles.tile([128, HR], F32, tag="RJBb", name="RJBb"),
        ]
        QJB = [
            singles.tile([128, HR], F32, tag="QJBa", name="QJBa"),
            singles.tile([128, HR], F32, tag="QJBb", name="QJBb"),
        ]
        RJB3 = singles.tile([128, NPAIR * 64], F32, tag="RJB3")
        QJB3 = singles.tile([128, NPAIR * 64], F32, tag="QJB3")
        for src, dst, dst3, nm in ((RI, RJB, RJB3, "ri"), (QI, QJB, QJB3, "qi")):
            scr = dram.tile([1, RPC * N], F32, tag=f"scr_{nm}")
            nc.gpsimd.dma_start(
                scr[:].rearrange("a (r c) -> (a r) c", r=RPC, c=N), src[:]
            )
            for h in range(2):
                nc.gpsimd.dma_start(
                    dst[h][:], scr[:, h * HR : (h + 1) * HR].broadcast_to([128, HR])
                )
            v = scr[:].rearrange("a (q two c) -> a q two c", q=NPAIR, two=2, c=N)
            nc.gpsimd.dma_start(
                dst3[0:64, :],
                v[:, :, 0:1, 128:N].broadcast_to([64, NPAIR, 1, 64]),
            )
            nc.gpsimd.dma_start(
                dst3[64:128, :],
                v[:, :, 1:2, 128:N].broadcast_to([64, NPAIR, 1, 64]),
            )

        # bf16 weight columns for the PE contractions (+ doubled B weights
        # standing in for the skipped mirror block)
        CDTb = {}
        for r0, P in CHUNKS:
            t = singles.tile([P, RPC], BF16, tag=f"cdtb_{r0}")
            nc.vector.tensor_copy(t[:], CDT[r0][:])
            CDTb[r0] = t
        CDTb2x = singles.tile([64, RPC], BF16, tag="cdtb2x")
        nc.vector.tensor_scalar_mul(CDTb2x[:], CDT[128][:], 2.0)

        # stacked-pair tables for the shared B pass
        X2B2 = singles.tile([128, N], F32, tag="x2b2")
        nc.gpsimd.dma_start(X2B2[0:64, :], X2[128][:])
        nc.gpsimd.dma_start(X2B2[64:128, :], X2[128][:])
        RIT3 = singles.tile([128, NPAIR], F32, tag="rit3")
        QIT3 = singles.tile([128, NPAIR], F32, tag="qit3")
        CDT3b = singles.tile([128, NPAIR], BF16, tag="cdt3b")
        for dst, srct in ((RIT3, RIT[128]), (QIT3, QIT[128]), (CDT3b, CDTb[128])):
            s3 = srct[:].rearrange("p (i two) -> p i two", two=2)
            nc.gpsimd.dma_start(dst[0:64, :], s3[:, :, 0:1])
            nc.gpsimd.dma_start(dst[64:128, :], s3[:, :, 1:2])
        QITn0 = singles.tile([128, RPC], F32, tag="qitn0")
        nc.vector.tensor_scalar_mul(QITn0[:], QIT[0][:], -1.0)
        QIT3n = singles.tile([128, NPAIR], F32, tag="qit3n")
        nc.vector.tensor_scalar_mul(QIT3n[:], QIT3[:], -1.0)

        FA = singles.tile([NFA, RPC], F32, tag="FA")

        # ---------------- main loop: 12 super-iterations of 2 row pairs ----
        # Software-pipelined: contractions for super-iter s are emitted after
        # the a-build/T5 of super-iter s+1, keeping ACT fed.
        PW = NFA * WF  # 2240: one pair's feature block
        pending = None  # (E, pairs) awaiting contraction

        def contract(E, pairs):
            for u, p in enumerate(pairs):
                i0, i1 = 2 * p, 2 * p + 1
                eo = u * PW
                vvs = {}
                for i in (i0, i1):
                    vvs[i] = psum3.tile(
                        [128, 3 * NFA], F32, tag="vps", name=f"vv_{i}"
                    )
                for f in range(NFA):
                    for t, i in ((0, i0), (1, i1)):
                        wk = CDTb[0][:, i : i + 1]
                        c = eo + f * WF + t * N
                        nc.tensor.matmul(
                            vvs[i][0:128, f : f + 1], E[:, c : c + 128], wk
                        )
                        nc.tensor.matmul(
                            vvs[i][0:64, NFA + f : NFA + f + 1],
                            E[:, c + 128 : c + N],
                            wk,
                        )
                    c = eo + f * WF + 2 * N
                    for h, i in enumerate((i0, i1)):
                        nc.tensor.matmul(
                            vvs[i][0:64, 2 * NFA + f : 2 * NFA + f + 1],
                            E[64 * h : 64 * (h + 1), c : c + 64],
                            CDT3b[64 * h : 64 * (h + 1), p : p + 1],
                        )
                for i in (i0, i1):
                    V5 = work4.tile(
                        [128, 3 * NFA], BF16, tag="V5", name=f"V5_{i}"
                    )
                    nc.vector.tensor_copy(V5[:], vvs[i][:])
                    fa3 = psum2.tile([NFA, 3], F32, tag="fa3", name=f"fa3_{i}")
                    nc.tensor.matmul(
                        fa3[:, 0:1], V5[0:128, 0:NFA], CDTb[0][:, i : i + 1]
                    )
                    nc.tensor.matmul(
                        fa3[:, 1:2], V5[0:64, NFA : 2 * NFA], CDTb2x[:, i : i + 1]
                    )
                    nc.tensor.matmul(
                        fa3[:, 2:3],
                        V5[0:64, 2 * NFA : 3 * NFA],
                        CDTb[128][:, i : i + 1],
                    )
                    nc.vector.tensor_reduce(
                        FA[:, i : i + 1],
                        fa3[:],
                        mybir.AxisListType.X,
                        mybir.AluOpType.add,
                    )

        for spi in range(NPAIR // 2):
            pairs = (2 * spi, 2 * spi + 1)

            T5 = work.tile([128, 2 * PW], BF16, tag="T5")
            for u, p in enumerate(pairs):
                i0, i1 = 2 * p, 2 * p + 1
                V3 = work4.tile([128, WF], F32, tag="V3", name=f"V3_{p}")
                Am3 = work4.tile([128, WF], F32, tag="Am3", name=f"Am3_{p}")

                for t in range(3):
                    if t < 2:
                        i = (i0, i1)[t]
                        h, base = i // (RPC // 2), (i % (RPC // 2)) * N
                        X2ap = X2[0][:]
                        rk = RIT[0][:, i : i + 1]
                        qk = QIT[0][:, i : i + 1]
                        qkn = QITn0[:, i : i + 1]
                        rjb = RJB[h][:, base : base + N]
                        qjb = QJB[h][:, base : base + N]
                        off, w = t * N, N
                    else:
                        X2ap = X2B2[:, 128:N]
                        rk = RIT3[:, p : p + 1]
                        qk = QIT3[:, p : p + 1]
                        qkn = QIT3n[:, p : p + 1]
                        rjb = RJB3[:, p * 64 : (p + 1) * 64]
                        qjb = QJB3[:, p * 64 : (p + 1) * 64]
                        off, w = 2 * N, 64
                    Vs = V3[:, off : off + w]
                    As = Am3[:, off : off + w]
                    # V = rk*X2 - qk; split across ACT/DVE to balance load
                    if t == 0:
                        nc.scalar.activation(Vs, X2ap, Identity, bias=qkn, scale=rk)
                    else:
                        nc.vector.tensor_scalar(Vs, X2ap, rk, qk, mult, subtract)
                    nc.vector.tensor_mul(As, Vs, rjb)
                    # As = (qjb*rk) - V*rjb = +a
                    nc.vector.scalar_tensor_tensor(As, qjb, rk, As, mult, subtract)

                for f in range(NFA):
                    nc.vector.tensor_scalar(
                        T5[:, u * PW + f * WF : u * PW + (f + 1) * WF],
                        Am3[:],
                        sc[f],
                        bc[f],
                        mult,
                        add,
                    )

            nc.scalar.activation(T5[:], T5[:], Square)  # bf16 in/out on ACT
            E = work.tile([128, 2 * PW], BF16, tag="E")
            nc.scalar.activation(E[:], T5[:], Exp, scale=-1.0)

            if pending is not None:
                contract(*pending)
            pending = (E, pairs)

        contract(*pending)

        outs = singles.tile([NFA, RPC], F32, tag="outs")
        nc.scalar.mul(outs[:], FA[:], 1.0 / (N * 12))
        nc.gpsimd.dma_start(out_e[:], outs[:])

    nc.finalize()
    return nc


def _get_nc(fp5, c5):
    key = (tuple(np.asarray(fp5).ravel().tolist()), tuple(np.asarray(c5).ravel().tolist()))
    if key not in _BUILT:
        # Am3 holds +a, so t_f = sqrt(100 c_f) * a - sqrt(100 c_f) * fp_f
        sc = [math.sqrt(100.0 * float(c)) for c in np.asarray(c5).ravel()]
        bc = [-s * float(f) for s, f in zip(sc, np.asarray(fp5).ravel())]
        _BUILT[key] = _build(sc, bc)
    return _BUILT[key]


def kernel(d, cd, fp, coeff):
    from concourse.bass_utils import run_bass_kernel_spmd

    d = np.asarray(d, dtype=np.float32)
    cd = np.asarray(cd, dtype=np.float32)
    cd0 = np.where(cd == 1.0, 0.0, cd).astype(np.float32)
    fp5 = np.asarray(fp, dtype=np.float32).reshape(NFA)
    c5 = np.asarray(coeff, dtype=np.float32).reshape(NFA)
    eye = np.eye(N, dtype=np.float32)

    in_maps = []
    for c in range(NCORES):
        b, i0 = c // 4, RPC * (c % 4)
        ds = d[b] + eye
        in_maps.append(
            {
                "d": np.ascontiguousarray(d[b]),
                "di": np.ascontiguousarray(d[b][i0 : i0 + RPC, :]),
                "dsi": np.ascontiguousarray(ds[i0 : i0 + RPC, :]),
                "dit": np.ascontiguousarray(d[b][:, i0 : i0 + RPC]),
                "dsit": np.ascontiguousarray(ds[:, i0 : i0 + RPC]),
                "cdt": np.ascontiguousarray(cd0[b].T[:, i0 : i0 + RPC]),
            }
        )

    global _last_in_maps, _last_res
    _last_in_maps = in_maps
    nc = _get_nc(fp5, c5)
    res = run_bass_kernel_spmd(nc, in_maps, core_ids=list(range(NCORES)))
    _last_res = res

    fa = np.zeros((2, N, NFA), np.float32)
    for c in range(NCORES):
        b, i0 = c // 4, RPC * (c % 4)
        fa[b, i0 : i0 + RPC, :] = res.results[c]["out"].T
    return fa


# revision 41
# speedup vs baseline: 1.6920x; 1.0441x over previous
"""Angle-feature extraction kernel for 8 TRN2 NeuronCores.

Math (per batch b, atom i):
  a[j,k]   = (d2_ij + d2_ik - d2_jk) / (2 d_ij d_ik)   (0 where den==0)
  fa[i,f]  = sum_jk exp(-100 c_f (a-fp_f)^2) w_ij w_ik / (N*12),  w = cd0 row i

Sharding: 8 cores x 48 (b,i)-rows (cores 0-3: b=0, 4-7: b=1).

Per core, rows are processed in PAIRS (i0,i1): k splits into a
128-partition A chunk per row plus one shared pass stacking both rows'
64-partition B chunks, so every op runs at full partition width. The
integrand is symmetric in j<->k, so the (k in B, j in A) block is skipped
entirely and the mirrored (k in A, j in B) contribution is counted twice
(doubled weights in the final contraction): per-feature width is
192+192+64 = 448 instead of 3*192.

Tiles are [k_partitions, j_free]; -a = (X2*rk - qk).rjb - rk.qjb with
rk/qk per-partition columns and rjb/qjb row-broadcasts fetched from
DMA-replicated tables (no PE involvement). fp/coeff are baked into the
program as immediates (recompiled per parameter set), so the feature
expansion t_f = s_f*(-a) + s_f*fp_f runs at the DVE 2x tensor_scalar
rate. One Square + one Exp (bf16 out) per pair on the scalar engine.
Both weighted contractions are PE matmuls with bf16 operands; partial
sums land in separate PSUM columns (cross-group PSUM accumulation
silently overwrites in this lowering) and a DVE reduce combines them.
Final scale 1/(N*12) rides the output copy.
"""

import sys

sys.path.insert(0, "/opt/trn_rl_repo")

import math
import numpy as np
from contextlib import ExitStack

N = 192
NFA = 5
NCORES = 8
RPC = 48  # (b,i) rows per core
NPAIR = RPC // 2
WF = 2 * N + 64  # 448: per-feature width (pass0 j:192, pass1 j:192, pass2 j:64)
SQRT_HALF = 0.7071067811865476

_BUILT = {}


def _build(sc, bc):
    """sc/bc: per-feature immediates  sc_f = -sqrt(100 c_f), bc_f = sc_f*fp_f."""
    from concourse import bacc, mybir, tile

    F32 = mybir.dt.float32
    BF16 = mybir.dt.bfloat16
    nc = bacc.Bacc(
        "TRN2", target_bir_lowering=False, debug=False, num_devices=NCORES
    )

    d_e = nc.declare_dram_parameter("d", [N, N], F32, isOutput=False)
    di_e = nc.declare_dram_parameter("di", [RPC, N], F32, isOutput=False)
    dsi_e = nc.declare_dram_parameter("dsi", [RPC, N], F32, isOutput=False)
    dit_e = nc.declare_dram_parameter("dit", [N, RPC], F32, isOutput=False)
    dsit_e = nc.declare_dram_parameter("dsit", [N, RPC], F32, isOutput=False)
    cdt_e = nc.declare_dram_parameter("cdt", [N, RPC], F32, isOutput=False)
    out_e = nc.declare_dram_parameter("out", [NFA, RPC], F32, isOutput=True)

    mult = mybir.AluOpType.mult
    subtract = mybir.AluOpType.subtract
    add = mybir.AluOpType.add
    Square = mybir.ActivationFunctionType.Square
    Exp = mybir.ActivationFunctionType.Exp
    Identity = mybir.ActivationFunctionType.Identity

    with tile.TileContext(nc) as tc, ExitStack() as ctx:
        singles = ctx.enter_context(tc.tile_pool(name="singles", bufs=1))
        work = ctx.enter_context(tc.tile_pool(name="work", bufs=3))
        work4 = ctx.enter_context(tc.tile_pool(name="work4", bufs=4))
        psum2 = ctx.enter_context(tc.tile_pool(name="psum2", bufs=4, space="PSUM"))
        psum3 = ctx.enter_context(tc.tile_pool(name="psum3", bufs=4, space="PSUM"))
        dram = ctx.enter_context(tc.tile_pool(name="dram", bufs=1, space="DRAM"))

        # ---------------- load inputs ----------------
        CHUNKS = [(0, 128), (128, 64)]
        D = {}
        CDT = {}
        for r0, P in CHUNKS:
            t = singles.tile([P, N], F32, tag=f"d_{r0}")
            nc.gpsimd.dma_start(t[:], d_e[r0 : r0 + P, :])
            D[r0] = t
            t = singles.tile([P, RPC], F32, tag=f"cdt_{r0}")
            nc.gpsimd.dma_start(t[:], cdt_e[r0 : r0 + P, :])
            CDT[r0] = t
        dI = singles.tile([RPC, N], F32, tag="dI")
        nc.gpsimd.dma_start(dI[:], di_e[:])
        dsI = singles.tile([RPC, N], F32, tag="dsI")
        nc.gpsimd.dma_start(dsI[:], dsi_e[:])
        dIT, dsIT = {}, {}
        for r0, P in CHUNKS:
            t = singles.tile([P, RPC], F32, tag=f"dit_{r0}")
            nc.gpsimd.dma_start(t[:], dit_e[r0 : r0 + P, :])
            dIT[r0] = t
            t = singles.tile([P, RPC], F32, tag=f"dsit_{r0}")
            nc.gpsimd.dma_start(t[:], dsit_e[r0 : r0 + P, :])
            dsIT[r0] = t

        # ---------------- derived matrices ----------------
        X2 = {}
        for r0, P in CHUNKS:
            t = singles.tile([P, N], F32, tag=f"x2_{r0}")
            nc.vector.tensor_mul(t[:], D[r0][:], D[r0][:])
            X2[r0] = t
        RI = singles.tile([RPC, N], F32, tag="RI")
        nc.vector.reciprocal(RI[:], dsI[:])
        nc.vector.tensor_scalar_mul(RI[:], RI[:], SQRT_HALF)
        QI = singles.tile([RPC, N], F32, tag="QI")
        nc.vector.tensor_mul(QI[:], dI[:], dI[:])
        nc.vector.tensor_mul(QI[:], QI[:], RI[:])

        # rk/qk per-partition column tables
        RIT = {}
        QIT = {}
        for r0, P in CHUNKS:
            rt = singles.tile([P, RPC], F32, tag=f"rit_{r0}")
            nc.vector.reciprocal(rt[:], dsIT[r0][:])
            nc.vector.tensor_scalar_mul(rt[:], rt[:], SQRT_HALF)
            RIT[r0] = rt
            qt = singles.tile([P, RPC], F32, tag=f"qit_{r0}")
            nc.vector.tensor_mul(qt[:], dIT[r0][:], dIT[r0][:])
            nc.vector.tensor_mul(qt[:], qt[:], rt[:])
            QIT[r0] = qt

        # rjb/qjb row-broadcast tables, DMA-replicated via a DRAM bounce.
        # RJB[p, i*192+j] = RI[i, j] for every partition p; the pair tables
        # RJB3 stack even rows on partitions 0-63 and odd rows on 64-127,
        # restricted to j in [128,192).
        NB = 4
        HR = RPC * N // NB
        H3 = NPAIR // 2 * 64
        RJB = [
            singles.tile([128, HR], F32, tag=f"RJB{h}", name=f"RJB{h}")
            for h in range(NB)
        ]
        QJB = [
            singles.tile([128, HR], F32, tag=f"QJB{h}", name=f"QJB{h}")
            for h in range(NB)
        ]
        RJB3 = [
            singles.tile([128, H3], F32, tag=f"RJB3{h}", name=f"RJB3{h}")
            for h in range(2)
        ]
        QJB3 = [
            singles.tile([128, H3], F32, tag=f"QJB3{h}", name=f"QJB3{h}")
            for h in range(2)
        ]
        for src, dst, dst3, nm in ((RI, RJB, RJB3, "ri"), (QI, QJB, QJB3, "qi")):
            scr = dram.tile([1, RPC * N], F32, tag=f"scr_{nm}")
            nc.gpsimd.dma_start(
                scr[:].rearrange("a (r c) -> (a r) c", r=RPC, c=N), src[:]
            )
            for h in range(NB):
                nc.gpsimd.dma_start(
                    dst[h][:], scr[:, h * HR : (h + 1) * HR].broadcast_to([128, HR])
                )
            v = scr[:].rearrange("a (q two c) -> a q two c", q=NPAIR, two=2, c=N)
            for h3 in range(2):
                q0 = h3 * (NPAIR // 2)
                for par, two in ((0, 0), (64, 1)):
                    nc.gpsimd.dma_start(
                        dst3[h3][par : par + 64, :],
                        v[:, q0 : q0 + NPAIR // 2, two : two + 1, 128:N].broadcast_to(
                            [64, NPAIR // 2, 1, 64]
                        ),
                    )

        # bf16 weight columns for the PE contractions (+ doubled B weights
        # standing in for the skipped mirror block)
        CDTb = {}
        for r0, P in CHUNKS:
            t = singles.tile([P, RPC], BF16, tag=f"cdtb_{r0}")
            nc.vector.tensor_copy(t[:], CDT[r0][:])
            CDTb[r0] = t
        CDTb2x = singles.tile([64, RPC], BF16, tag="cdtb2x")
        nc.vector.tensor_scalar_mul(CDTb2x[:], CDT[128][:], 2.0)

        # stacked-pair tables for the shared B pass
        X2B2 = singles.tile([128, N], F32, tag="x2b2")
        nc.gpsimd.dma_start(X2B2[0:64, :], X2[128][:])
        nc.gpsimd.dma_start(X2B2[64:128, :], X2[128][:])
        RIT3 = singles.tile([128, NPAIR], F32, tag="rit3")
        QIT3 = singles.tile([128, NPAIR], F32, tag="qit3")
        CDT3b = singles.tile([128, NPAIR], BF16, tag="cdt3b")
        for dst, srct in ((RIT3, RIT[128]), (QIT3, QIT[128]), (CDT3b, CDTb[128])):
            s3 = srct[:].rearrange("p (i two) -> p i two", two=2)
            nc.gpsimd.dma_start(dst[0:64, :], s3[:, :, 0:1])
            nc.gpsimd.dma_start(dst[64:128, :], s3[:, :, 1:2])
        QITn0 = singles.tile([128, RPC], F32, tag="qitn0")
        nc.vector.tensor_scalar_mul(QITn0[:], QIT[0][:], -1.0)
        QIT3n = singles.tile([128, NPAIR], F32, tag="qit3n")
        nc.vector.tensor_scalar_mul(QIT3n[:], QIT3[:], -1.0)

        FA = singles.tile([NFA, RPC], F32, tag="FA")

        # ---------------- main loop: 12 super-iterations of 2 row pairs ----
        # Software-pipelined: contractions for super-iter s are emitted after
        # the a-build/T5 of super-iter s+1, keeping ACT fed.
        PW = NFA * WF  # 2240: one pair's feature block
        pending = None  # (E, pairs) awaiting contraction

        def contract(E, pairs):
            for u, p in enumerate(pairs):
                i0, i1 = 2 * p, 2 * p + 1
                eo = u * PW
                vvs = {}
                for i in (i0, i1):
                    vvs[i] = psum3.tile(
                        [128, 3 * NFA], F32, tag="vps", name=f"vv_{i}"
                    )
                for f in range(NFA):
                    for t, i in ((0, i0), (1, i1)):
                        wk = CDTb[0][:, i : i + 1]
                        c = eo + f * WF + t * N
                        nc.tensor.matmul(
                            vvs[i][0:128, f : f + 1], E[:, c : c + 128], wk
                        )
                        nc.tensor.matmul(
                            vvs[i][0:64, NFA + f : NFA + f + 1],
                            E[:, c + 128 : c + N],
                            wk,
                        )
                    c = eo + f * WF + 2 * N
                    for h, i in enumerate((i0, i1)):
                        nc.tensor.matmul(
                            vvs[i][0:64, 2 * NFA + f : 2 * NFA + f + 1],
                            E[64 * h : 64 * (h + 1), c : c + 64],
                            CDT3b[64 * h : 64 * (h + 1), p : p + 1],
                        )
                for i in (i0, i1):
                    V5 = work4.tile(
                        [128, 3 * NFA], BF16, tag="V5", name=f"V5_{i}"
                    )
                    nc.vector.tensor_copy(V5[:], vvs[i][:])
                    fa3 = psum2.tile([NFA, 3], F32, tag="fa3", name=f"fa3_{i}")
                    nc.tensor.matmul(
                        fa3[:, 0:1], V5[0:128, 0:NFA], CDTb[0][:, i : i + 1]
                    )
                    nc.tensor.matmul(
                        fa3[:, 1:2], V5[0:64, NFA : 2 * NFA], CDTb2x[:, i : i + 1]
                    )
                    nc.tensor.matmul(
                        fa3[:, 2:3],
                        V5[0:64, 2 * NFA : 3 * NFA],
                        CDTb[128][:, i : i + 1],
                    )
                    nc.vector.tensor_reduce(
                        FA[:, i : i + 1],
                        fa3[:],
                        mybir.AxisListType.X,
                        mybir.AluOpType.add,
                    )

        for spi in range(NPAIR // 2):
            pairs = (2 * spi, 2 * spi + 1)

            T5 = work.tile([128, 2 * PW], BF16, tag="T5")
            for u, p in enumerate(pairs):
                i0, i1 = 2 * p, 2 * p + 1
                V3 = work4.tile([128, WF], F32, tag="V3", name=f"V3_{p}")
                Am3 = work4.tile([128, WF], F32, tag="Am3", name=f"Am3_{p}")

                for t in range(3):
                    if t < 2:
                        i = (i0, i1)[t]
                        h, base = i // (RPC // 4), (i % (RPC // 4)) * N
                        X2ap = X2[0][:]
                        rk = RIT[0][:, i : i + 1]
                        qk = QIT[0][:, i : i + 1]
                        qkn = QITn0[:, i : i + 1]
                        rjb = RJB[h][:, base : base + N]
                        qjb = QJB[h][:, base : base + N]
                        off, w = t * N, N
                    else:
                        X2ap = X2B2[:, 128:N]
                        rk = RIT3[:, p : p + 1]
                        qk = QIT3[:, p : p + 1]
                        qkn = QIT3n[:, p : p + 1]
                        h3, p3 = p // (NPAIR // 2), p % (NPAIR // 2)
                        rjb = RJB3[h3][:, p3 * 64 : (p3 + 1) * 64]
                        qjb = QJB3[h3][:, p3 * 64 : (p3 + 1) * 64]
                        off, w = 2 * N, 64
                    Vs = V3[:, off : off + w]
                    As = Am3[:, off : off + w]
                    # V = rk*X2 - qk; split across ACT/DVE to balance load
                    if t == 0:
                        nc.scalar.activation(Vs, X2ap, Identity, bias=qkn, scale=rk)
                    else:
                        nc.vector.tensor_scalar(Vs, X2ap, rk, qk, mult, subtract)
                    nc.vector.tensor_mul(As, Vs, rjb)
                    # As = (qjb*rk) - V*rjb = +a
                    nc.vector.scalar_tensor_tensor(As, qjb, rk, As, mult, subtract)

                for f in range(NFA):
                    nc.vector.tensor_scalar(
                        T5[:, u * PW + f * WF : u * PW + (f + 1) * WF],
                        Am3[:],
                        sc[f],
                        bc[f],
                        mult,
                        add,
                    )

            E = work.tile([128, 2 * PW], BF16, tag="E")
            for u in range(2):
                sl = slice(u * PW, (u + 1) * PW)
                nc.scalar.activation(T5[:, sl], T5[:, sl], Square)
                nc.scalar.activation(E[:, sl], T5[:, sl], Exp, scale=-1.0)

            if pending is not None:
                contract(*pending)
            pending = (E, pairs)

        contract(*pending)

        outs = singles.tile([NFA, RPC], F32, tag="outs")
        nc.scalar.mul(outs[:], FA[:], 1.0 / (N * 12))
        nc.gpsimd.dma_start(out_e[:], outs[:])

    nc.finalize()
    return nc


def _get_nc(fp5, c5):
    key = (tuple(np.asarray(fp5).ravel().tolist()), tuple(np.asarray(c5).ravel().tolist()))
    if key not in _BUILT:
        # Am3 holds +a, so t_f = sqrt(100 c_f) * a - sqrt(100 c_f) * fp_f
        sc = [math.sqrt(100.0 * float(c)) for c in np.asarray(c5).ravel()]
        bc = [-s * float(f) for s, f in zip(sc, np.asarray(fp5).ravel())]
        _BUILT[key] = _build(sc, bc)
    return _BUILT[key]


def kernel(d, cd, fp, coeff):
    from concourse.bass_utils import run_bass_kernel_spmd

    d = np.asarray(d, dtype=np.float32)
    cd = np.asarray(cd, dtype=np.float32)
    cd0 = np.where(cd == 1.0, 0.0, cd).astype(np.float32)
    fp5 = np.asarray(fp, dtype=np.float32).reshape(NFA)
    c5 = np.asarray(coeff, dtype=np.float32).reshape(NFA)
    eye = np.eye(N, dtype=np.float32)

    in_maps = []
    for c in range(NCORES):
        b, i0 = c // 4, RPC * (c % 4)
        ds = d[b] + eye
        in_maps.append(
            {
                "d": np.ascontiguousarray(d[b]),
                "di": np.ascontiguousarray(d[b][i0 : i0 + RPC, :]),
                "dsi": np.ascontiguousarray(ds[i0 : i0 + RPC, :]),
                "dit": np.ascontiguousarray(d[b][:, i0 : i0 + RPC]),
                "dsit": np.ascontiguousarray(ds[:, i0 : i0 + RPC]),
                "cdt": np.ascontiguousarray(cd0[b].T[:, i0 : i0 + RPC]),
            }
        )

    global _last_in_maps, _last_res
    _last_in_maps = in_maps
    nc = _get_nc(fp5, c5)
    res = run_bass_kernel_spmd(nc, in_maps, core_ids=list(range(NCORES)))
    _last_res = res

    fa = np.zeros((2, N, NFA), np.float32)
    for c in range(NCORES):
        b, i0 = c // 4, RPC * (c % 4)
        fa[b, i0 : i0 + RPC, :] = res.results[c]["out"].T
    return fa
